# revision 27
# baseline (speedup 1.0000x reference)
"""AttentionPairBias kernel for Trainium2, 8-core SPMD — v2.

Math (per batch=1):
  pn        = LayerNorm(pairwise) * gamma + beta                  [N, N, 128]
  attn_bias = einsum('ijp,ph->hij', pn, W_bias)                   [16, N, N]
  q,k,v     = single @ Wq/Wk/Wv  (split into 16 heads of 64)
  scores    = q k^T / sqrt(64) + attn_bias ; attn = softmax_j
  o         = attn @ v ; out = (o * sigmoid(single@Wg + bg)) @ Wo [N, 1024]

Sharding: rows of i (queries) across 8 cores; k/v compute replicated.

v2 design vs v1:
- The pairwise tensor is pre-transposed ON HOST to [p, i, j] layout (bf16),
  removing all on-chip PE transposes and the 16.7M-element PSUM->SBUF
  copies of v1. DMA ships 32MB/core instead of 64MB-read.
- LayerNorm mean correction is folded into the weights:
      W' = gamma*W_bias - colsum(gamma*W_bias)/128
  so bias = rsqrt(var+eps) * (x @ W'), turning the per-element affine
  correction into a single broadcast multiply.
- Per j-column: matmul(x_tile[p,i] stationary, [W'|ones] moving) gives
  bias products AND row-sums in one shot; a second 1-column matmul with a
  squared copy of the tile gives sumsq (for variance).
- softmax 1/sum scaling folded into the attention transpose by using
  diag(1/sum) instead of the identity matrix.
- qkvg projections interleaved between pairwise chunks so the PE stays
  busy while DMA streams the big tensor; all DMAs issued from SP.
"""

import numpy as np
import ml_dtypes

import concourse.bacc as bacc
import concourse.bass as bass
import concourse.tile as tile
import concourse.mybir as mybir
from concourse.bass_utils import run_bass_kernel_spmd
from concourse.masks import make_identity

N, DIM, HEADS, DHEAD, DPAIR = 1024, 1024, 16, 64, 128
NCORES = 8
IBLK = N // NCORES  # 128
EPS = 1e-5
JW = 32            # j-columns per pairwise DMA chunk
NCH = N // JW      # 32 chunks
JG = 16            # j-columns per PSUM group

F32 = mybir.dt.float32
BF16 = mybir.dt.bfloat16
AX = mybir.AxisListType
AF = mybir.ActivationFunctionType
BFNP = ml_dtypes.bfloat16


def _swap_free(ap):
    """Swap the two free dims of a 3D AP (iteration-order change)."""
    l = list(ap.ap)
    assert len(l) == 3
    return bass.AP(tensor=ap.tensor, offset=ap.offset, ap=[l[0], l[2], l[1]])


def _insert_bcast(ap, count, pos):
    """Insert a zero-stride broadcast dim of length `count` at free-dim
    position `pos` (0 = right after the partition dim)."""
    l = list(ap.ap)
    l.insert(1 + pos, [0, count])
    return bass.AP(tensor=ap.tensor, offset=ap.offset, ap=l)


def build_program(reps=1, sq_act_mod=4, **_):
    """sq_act_mod: every sq_act_mod-th square group runs on ACT (rest DVE)."""
    nc = bacc.Bacc("TRN2", target_bir_lowering=False, debug=False)

    ptb = nc.dram_tensor("ptb", [128, NCH, IBLK, JW], BF16, kind="ExternalInput")
    sT = nc.dram_tensor("sT", [DIM, N], BF16, kind="ExternalInput")
    sTi = nc.dram_tensor("sTi", [DIM, IBLK], BF16, kind="ExternalInput")
    wq = nc.dram_tensor("wq", [DIM, DIM], BF16, kind="ExternalInput")
    wk = nc.dram_tensor("wk", [DIM, DIM], BF16, kind="ExternalInput")
    wv = nc.dram_tensor("wv", [DIM, DIM], BF16, kind="ExternalInput")
    wg = nc.dram_tensor("wg", [DIM, DIM], BF16, kind="ExternalInput")
    wo = nc.dram_tensor("wo", [DIM, DIM], BF16, kind="ExternalInput")
    w17 = nc.dram_tensor("w17", [DPAIR, 17], BF16, kind="ExternalInput")
    bgt = nc.dram_tensor("bgt", [128, 8], F32, kind="ExternalInput")
    out = nc.dram_tensor("out", [IBLK, DIM], F32, kind="ExternalOutput")

    CT = DIM // 128  # 8 contraction tiles

    with tile.TileContext(nc) as tc:
        with tc.tile_pool(name="consts", bufs=1) as consts, \
             tc.tile_pool(name="persist", bufs=1) as pers:
            ident = consts.tile([128, 128], BF16, tag="ident", name="ident")
            make_identity(nc, ident)
            ones1 = consts.tile([128, 1], BF16, tag="ones1", name="ones1")
            nc.vector.memset(ones1, 1.0)
            zero1 = consts.tile([128, 1], F32, tag="zero1", name="zero1")
            nc.vector.memset(zero1, 0.0)
            eps4 = consts.tile([128, 1], F32, tag="eps4", name="eps4")
            nc.vector.memset(eps4, EPS)
            w17_sb = consts.tile([DPAIR, 17], BF16, tag="w17", name="w17")
            nc.sync.dma_start(out=w17_sb, in_=w17[:, :])
            bgt_sb = consts.tile([128, 8], F32, tag="bgt", name="bgt")
            nc.sync.dma_start(out=bgt_sb, in_=bgt[:, :])

            for _rep in range(reps):
                # persistent tensors
                kT = [pers.tile([128, N], BF16, tag=f"kT{t}", name=f"kT{t}") for t in range(8)]
                vsb = [pers.tile([128, DIM], BF16, tag=f"v{t}", name=f"v{t}") for t in range(8)]
                qT = [pers.tile([128, IBLK], BF16, tag=f"qT{t}", name=f"qT{t}") for t in range(8)]
                gT = [pers.tile([128, IBLK], F32, tag=f"gT{t}", name=f"gT{t}") for t in range(8)]
                bias_h = pers.tile([128, HEADS, N], BF16, tag="biasH", name="biasH")
                wo_sb = pers.tile([128, CT, DIM], BF16, tag="wo", name="wo")

                with tc.tile_pool(name="pb", bufs=1) as pb, \
                     tc.tile_pool(name="psB", bufs=2, space="PSUM") as psB, \
                     tc.tile_pool(name="pa", bufs=1) as pa, \
                     tc.tile_pool(name="psA", bufs=2, space="PSUM") as psA:
                    sums = pb.tile([128, N], F32, tag="sums", name="sums")
                    sumsq = pb.tile([128, N], F32, tag="sumsq", name="sumsq")
                    rA = pb.tile([128, N], BF16, tag="rA", name="rA")

                    # ---- phase A inputs (SP queue) ----
                    def load8(dst, src, ncols):
                        for ct in range(CT):
                            nc.sync.dma_start(
                                out=dst[:, ct, :],
                                in_=src[ct * 128:(ct + 1) * 128, :ncols])

                    # first pairwise chunk goes out before anything else so
                    # phase B can start immediately
                    x0 = pb.tile([128, IBLK, JW], BF16, tag="x", bufs=3, name="x")
                    nc.sync.dma_start(out=x0, in_=ptb[:, 0, :, :])
                    si_sb = pa.tile([128, CT, IBLK], BF16, tag="si", name="si")
                    load8(si_sb, sTi, IBLK)
                    wA = pa.tile([128, CT, DIM], BF16, tag="wA", name="wA")
                    load8(wA, wq, DIM)

                    # ---- phase A unit emitters ----
                    s_sb = pa.tile([128, CT, N], BF16, tag="s", name="s")

                    def unit_q(t, eng):
                        ps = psA.tile([128, IBLK], F32, tag="mmA", name="mmA", bufs=2)
                        for ct in range(CT):
                            nc.tensor.matmul(
                                ps, wA[:, ct, t * 128:(t + 1) * 128], si_sb[:, ct, :],
                                start=(ct == 0), stop=(ct == CT - 1))
                        _copy(eng, qT[t], ps)

                    def unit_g(t, eng):
                        ps = psA.tile([128, IBLK], F32, tag="mmA", name="mmA", bufs=2)
                        for ct in range(CT):
                            nc.tensor.matmul(
                                ps, wB[:, ct, t * 128:(t + 1) * 128], si_sb[:, ct, :],
                                start=(ct == 0), stop=(ct == CT - 1))
                        nc.scalar.activation(out=gT[t], in_=ps, func=AF.Sigmoid,
                                             bias=bgt_sb[:, t:t + 1], scale=1.0)

                    def unit_k(t, jh, eng):
                        ps = psA.tile([128, 512], F32, tag="mmA2", name="mmA2", bufs=2)
                        for ct in range(CT):
                            nc.tensor.matmul(
                                ps, wA[:, ct, t * 128:(t + 1) * 128],
                                s_sb[:, ct, jh * 512:(jh + 1) * 512],
                                start=(ct == 0), stop=(ct == CT - 1))
                        _copy(eng, kT[t][:, jh * 512:(jh + 1) * 512], ps)

                    def unit_v(t, vh, eng):
                        ps = psA.tile([128, 512], F32, tag="mmA2", name="mmA2", bufs=2)
                        for ct in range(CT):
                            nc.tensor.matmul(
                                ps, s_sb[:, ct, t * 128:(t + 1) * 128],
                                wB[:, ct, vh * 512:(vh + 1) * 512],
                                start=(ct == 0), stop=(ct == CT - 1))
                        _copy(eng, vsb[t][:, vh * 512:(vh + 1) * 512], ps)

                    def _copy(eng, dst, src):
                        # PSUM sources: only DVE/ACT may touch PSUM
                        if eng % 2 == 0:
                            nc.vector.tensor_copy(out=dst, in_=src)
                        else:
                            nc.scalar.copy(out=dst, in_=src)

                    units = []
                    for t in range(8):
                        units.append(lambda t=t: unit_q(t, t % 2))
                    for t in range(8):
                        units.append(lambda t=t: unit_g(t, t % 2))
                    for t in range(8):
                        units.append(lambda t=t: unit_k(t, 0, 0))
                        units.append(lambda t=t: unit_k(t, 1, 1))
                    for t in range(8):
                        units.append(lambda t=t: unit_v(t, 0, 0))
                        units.append(lambda t=t: unit_v(t, 1, 1))
                    n_units = len(units)  # 48
                    unit_idx = 0

                    # emission checkpoints: after chunk jb, how many units
                    # should have been emitted (q/g early, k/v from chunk 3)
                    quota = [(jb + 1) * 16 // 6 if jb < 6 else
                             16 + ((jb - 5) * 32 + 25) // 26
                             for jb in range(NCH)]
                    quota[-1] = n_units

                    def emit_stats(j0, j1):
                        # stats post-pass runs on Pool (SBUF-only data)
                        sl = slice(j0, j1)
                        w = j1 - j0
                        mu = pb.tile([128, 256], F32, tag="mu", name="mu", bufs=2)[:, :w]
                        v4 = pb.tile([128, 256], F32, tag="v4", name="v4", bufs=2)[:, :w]
                        d = pb.tile([128, 256], F32, tag="d", name="d", bufs=2)[:, :w]
                        nc.gpsimd.tensor_scalar_mul(out=mu, in0=sums[:, sl],
                                                    scalar1=1.0 / DPAIR)
                        nc.gpsimd.tensor_scalar_mul(out=v4, in0=sumsq[:, sl],
                                                    scalar1=1.0 / DPAIR)
                        nc.gpsimd.tensor_mul(out=d, in0=mu, in1=mu)
                        nc.gpsimd.tensor_sub(out=v4, in0=v4, in1=d)
                        nc.scalar.activation(out=v4, in_=v4, func=AF.Sqrt,
                                             bias=eps4[:, 0:1], scale=1.0)
                        with nc.allow_low_precision(reason="rsqrt to bf16 is plenty for a softmax bias"):
                            nc.vector.reciprocal(out=rA[:, sl], in_=v4)
                        rA_b = _insert_bcast(rA[:, sl], HEADS, 0)
                        nc.vector.tensor_mul(out=bias_h[:, :, sl],
                                             in0=bias_h[:, :, sl], in1=rA_b)

                    # ---- phase B chunk loop with A units interleaved ----
                    for jb in range(NCH):
                        if jb == 0:
                            x = x0
                        else:
                            x = pb.tile([128, IBLK, JW], BF16, tag="x", bufs=3, name="x")
                            nc.sync.dma_start(out=x, in_=ptb[:, jb, :, :])
                        # stagger the remaining input DMAs between chunks
                        if jb == 0:
                            wB = pa.tile([128, CT, DIM], BF16, tag="wB", name="wB")
                            load8(wB, wg, DIM)
                        elif jb == 2:
                            load8(s_sb, sT, N)
                        elif jb == 4:
                            load8(wA, wk, DIM)
                        elif jb == 8:
                            load8(wB, wv, DIM)
                        elif jb == 12:
                            load8(wo_sb, wo, DIM)

                        # squares first (rotated DVE/ACT/Pool) so sq matmuls
                        # aren't waiting when PE reaches them
                        xs_t = []
                        for g in range(JW // JG):
                            j0 = g * JG
                            xs = pb.tile([128, IBLK, JG], BF16, tag="xsq",
                                         bufs=4, name="xsq")
                            xs_t.append(xs)
                            gi = (jb * (JW // JG) + g) % 16
                            if gi in (0, 6, 12):
                                nc.scalar.activation(out=xs, in_=x[:, :, j0:j0 + JG],
                                                     func=AF.Square)
                            elif gi in (2, 8, 14):
                                nc.gpsimd.tensor_mul(out=xs, in0=x[:, :, j0:j0 + JG],
                                                     in1=x[:, :, j0:j0 + JG])
                            else:
                                nc.vector.tensor_mul(out=xs, in0=x[:, :, j0:j0 + JG],
                                                     in1=x[:, :, j0:j0 + JG])
                        for g in range(JW // JG):
                            j0 = jb * JW + g * JG
                            gi = jb * (JW // JG) + g
                            pp = psB.tile([128, JG, 17], F32, tag="pp", bufs=2,
                                          name="pp")
                            for jj in range(JG):
                                nc.tensor.matmul(pp[:, jj, :], x[:, :, g * JG + jj],
                                                 w17_sb, start=True, stop=True)
                            sq = psB.tile([128, JG], F32, tag="sq", bufs=2, name="sq")
                            for jj in range(JG):
                                nc.tensor.matmul(sq[:, jj:jj + 1], xs_t[g][:, :, jj],
                                                 ones1, start=True, stop=True)
                            # copies: bias block + stats rotate DVE/ACT
                            bsl = bias_h[:, :, j0:j0 + JG]
                            _copy(gi % 2, _swap_free(bsl), pp[:, :, 0:16])
                            _copy(gi + 1, sums[:, j0:j0 + JG], pp[:, :, 16])
                            _copy(gi + 1, sumsq[:, j0:j0 + JG], sq)

                        if jb == NCH - 3:
                            # preload the Exp act table before the tail chain
                            dummy = pb.tile([128, 1], F32, tag="dummy", name="dummy")
                            nc.scalar.activation(out=dummy, in_=eps4, func=AF.Exp,
                                                 bias=zero1[:, 0:1], scale=1.0)
                        if jb % (NCH // 4) == NCH // 4 - 1 and jb != NCH - 1:
                            q = jb // (NCH // 4)
                            emit_stats(q * 256, q * 256 + 256)
                        elif jb == NCH - 2:
                            emit_stats(768, 896)
                        elif jb == NCH - 1:
                            emit_stats(896, 1024)
                        while unit_idx < quota[jb]:
                            units[unit_idx]()
                            unit_idx += 1

                # ---------------- Phase C: attention ------------------------
                with tc.tile_pool(name="pc", bufs=1) as pc, \
                     tc.tile_pool(name="psC", bufs=2, space="PSUM") as psC:
                    og = [pc.tile([128, IBLK], BF16, tag=f"og{t}", name=f"og{t}") for t in range(8)]

                    # Software-pipelined head loop: head h's scores+softmax are
                    # emitted one iteration ahead of head h's attn@v, so the
                    # PE never sits idle waiting for exp/normalize and stays
                    # out of the low p-state.
                    ot_ps = {}
                    attn_t = {}

                    def emit_scores(h):
                        t = h // 2
                        off = 64 * (h % 2)
                        # bias pre-written into PSUM via an identity matmul;
                        # the qk matmul accumulates on top with start=False,
                        # and exp reads straight from PSUM — no separate add.
                        ps = psC.tile([128, N], F32, tag="sc", bufs=2, name="sc")
                        for jh in range(2):
                            nc.tensor.matmul(
                                ps[:, jh * 512:(jh + 1) * 512], ident,
                                bias_h[:, h, jh * 512:(jh + 1) * 512],
                                start=True, stop=False, skip_group_check=True)
                            nc.tensor.matmul(
                                ps[:, jh * 512:(jh + 1) * 512],
                                qT[t][off:off + 64, :],
                                kT[t][off:off + 64, jh * 512:(jh + 1) * 512],
                                start=False, stop=True, skip_group_check=True)
                        # scores are O(10) here: exp() without max-subtraction
                        # is safe in f32 and softmax is shift-invariant.
                        ssum = pc.tile([128, 1], F32, tag="ssum", bufs=3, name="ssum")
                        attn = pc.tile([128, N], BF16, tag="attn", bufs=3, name="attn")
                        nc.scalar.activation(out=attn, in_=ps, func=AF.Exp,
                                             bias=zero1[:, 0:1], scale=1.0,
                                             accum_out=ssum)
                        rs = pc.tile([128, 1], F32, tag="rs", bufs=3, name="rs")
                        nc.vector.reciprocal(out=rs, in_=ssum)
                        nc.vector.tensor_scalar_mul(out=attn, in0=attn, scalar1=rs)
                        attn_t[h] = attn

                    def emit_av(h):
                        t = h // 2
                        off = 64 * (h % 2)
                        attn = attn_t.pop(h)
                        if h % 2 == 0:
                            ot_ps[t] = psC.tile([128, IBLK], F32, tag="ot", bufs=1, name="ot")
                        for j4 in range(2):
                            pT = psC.tile([128, 4, 128], BF16, tag="pT", bufs=2, name="pT")
                            for jj in range(4):
                                jt = j4 * 4 + jj
                                nc.tensor.transpose(
                                    pT[:, jj, :],
                                    attn[:, jt * 128:(jt + 1) * 128], ident)
                            aT = pc.tile([128, 4, 128], BF16, tag="aT", bufs=4, name="aT")
                            if (h + j4) % 2 == 0:
                                nc.vector.tensor_copy(out=aT.bitcast(F32),
                                                      in_=pT.bitcast(F32))
                            else:
                                nc.scalar.copy(out=aT.bitcast(F32),
                                               in_=pT.bitcast(F32))
                            for jj in range(4):
                                jt = j4 * 4 + jj
                                nc.tensor.matmul(
                                    ot_ps[t][off:off + 64, :],
                                    vsb[jt][:, h * 64:(h + 1) * 64], aT[:, jj, :],
                                    start=(jt == 0), stop=(jt == 7))
                        if h % 2 == 1:
                            nc.vector.tensor_mul(out=og[t], in0=ot_ps.pop(t), in1=gT[t])

                    for h in range(HEADS + 1):
                        if h < HEADS:
                            emit_scores(h)
                        if h >= 1:
                            emit_av(h - 1)

                    # out = og^T @ Wo
                    out_sb = pc.tile([128, DIM], F32, tag="out_sb", name="out_sb")
                    for eh in range(2):
                        ps = psC.tile([128, 512], F32, tag="po", bufs=1, name="po")
                        for t in range(8):
                            nc.tensor.matmul(
                                ps, og[t], wo_sb[:, t, eh * 512:(eh + 1) * 512],
                                start=(t == 0), stop=(t == 7))
                        nc.scalar.copy(out=out_sb[:, eh * 512:(eh + 1) * 512], in_=ps)
                    nc.sync.dma_start(out=out[:, :], in_=out_sb)

    nc.compile()
    return nc


_CACHE = {}


def _prep_inputs(single_repr, pairwise_repr, ln_gamma, ln_beta, W_bias,
                 Wq, Wk, Wv, Wg, bg, Wo):
    sr = np.asarray(single_repr, np.float32).reshape(N, DIM)
    pw = np.asarray(pairwise_repr, np.float32).reshape(N, N, DPAIR)
    gamma = np.asarray(ln_gamma, np.float32)
    Wb = np.asarray(W_bias, np.float32)
    weff = gamma[:, None] * Wb                                   # [128, 16]
    wp = weff - weff.sum(0, keepdims=True) / DPAIR               # mean folded
    w17 = np.concatenate([wp, np.ones((DPAIR, 1), np.float32)], axis=1)
    sT_np = np.ascontiguousarray(sr.T).astype(BFNP)              # [DIM, N]
    scale = DHEAD ** -0.5
    pwb = pw.astype(BFNP)                                        # [N, N, 128]
    common = {
        "sT": sT_np,
        "wq": (np.asarray(Wq, np.float32) * scale).astype(BFNP),
        "wk": np.asarray(Wk, np.float32).astype(BFNP),
        "wv": np.asarray(Wv, np.float32).astype(BFNP),
        "wg": np.asarray(Wg, np.float32).astype(BFNP),
        "wo": np.asarray(Wo, np.float32).astype(BFNP),
        "w17": w17.astype(BFNP),
        "bgt": np.ascontiguousarray(
            np.asarray(bg, np.float32).reshape(8, 128).T),
    }
    in_maps = []
    for c in range(NCORES):
        m = dict(common)
        # [i, j, p] -> [p, jb, i, jw]
        blk = pwb[c * IBLK:(c + 1) * IBLK]                       # [128i, N, 128p]
        m["ptb"] = np.ascontiguousarray(
            blk.transpose(2, 0, 1).reshape(128, IBLK, NCH, JW)
               .transpose(0, 2, 1, 3))
        m["sTi"] = np.ascontiguousarray(sT_np[:, c * IBLK:(c + 1) * IBLK])
        in_maps.append(m)
    return in_maps


def kernel(single_repr, pairwise_repr, ln_gamma, ln_beta, W_bias,
           Wq, Wk, Wv, Wg, bg, Wo, _trace=False):
    if "nc" not in _CACHE:
        _CACHE["nc"] = build_program()
    nc = _CACHE["nc"]
    in_maps = _prep_inputs(single_repr, pairwise_repr, ln_gamma, ln_beta,
                           W_bias, Wq, Wk, Wv, Wg, bg, Wo)
    res = run_bass_kernel_spmd(nc, in_maps, core_ids=list(range(NCORES)),
                               trace=_trace)
    out = np.concatenate([res.results[c]["out"] for c in range(NCORES)], axis=0)
    if _trace:
        kernel.last_result = res
    return out.reshape(1, N, DIM).astype(np.float32)


# revision 28
# speedup vs baseline: 1.2430x; 1.2430x over previous
"""AttentionPairBias kernel for Trainium2, 8-core SPMD — v2.

Math (per batch=1):
  pn        = LayerNorm(pairwise) * gamma + beta                  [N, N, 128]
  attn_bias = einsum('ijp,ph->hij', pn, W_bias)                   [16, N, N]
  q,k,v     = single @ Wq/Wk/Wv  (split into 16 heads of 64)
  scores    = q k^T / sqrt(64) + attn_bias ; attn = softmax_j
  o         = attn @ v ; out = (o * sigmoid(single@Wg + bg)) @ Wo [N, 1024]

Sharding: rows of i (queries) across 8 cores; k/v compute replicated.

v2 design vs v1:
- The pairwise tensor is pre-transposed ON HOST to [p, i, j] layout (bf16),
  removing all on-chip PE transposes and the 16.7M-element PSUM->SBUF
  copies of v1. DMA ships 32MB/core instead of 64MB-read.
- LayerNorm mean correction is folded into the weights:
      W' = gamma*W_bias - colsum(gamma*W_bias)/128
  so bias = rsqrt(var+eps) * (x @ W'), turning the per-element affine
  correction into a single broadcast multiply.
- Per j-column: matmul(x_tile[p,i] stationary, [W'|ones] moving) gives
  bias products AND row-sums in one shot; a second 1-column matmul with a
  squared copy of the tile gives sumsq (for variance).
- softmax 1/sum scaling folded into the attention transpose by using
  diag(1/sum) instead of the identity matrix.
- qkvg projections interleaved between pairwise chunks so the PE stays
  busy while DMA streams the big tensor; all DMAs issued from SP.
"""

import numpy as np
import ml_dtypes

import concourse.bacc as bacc
import concourse.bass as bass
import concourse.tile as tile
import concourse.mybir as mybir
from concourse.bass_utils import run_bass_kernel_spmd
from concourse.masks import make_identity

N, DIM, HEADS, DHEAD, DPAIR = 1024, 1024, 16, 64, 128
NCORES = 8
IBLK = N // NCORES  # 128
EPS = 1e-5
JW = 32            # j-columns per pairwise DMA chunk
NCH = N // JW      # 32 chunks
JG = 16            # j-columns per PSUM group

F32 = mybir.dt.float32
BF16 = mybir.dt.bfloat16
AX = mybir.AxisListType
AF = mybir.ActivationFunctionType
BFNP = ml_dtypes.bfloat16


def _swap_free(ap):
    """Swap the two free dims of a 3D AP (iteration-order change)."""
    l = list(ap.ap)
    assert len(l) == 3
    return bass.AP(tensor=ap.tensor, offset=ap.offset, ap=[l[0], l[2], l[1]])


def _insert_bcast(ap, count, pos):
    """Insert a zero-stride broadcast dim of length `count` at free-dim
    position `pos` (0 = right after the partition dim)."""
    l = list(ap.ap)
    l.insert(1 + pos, [0, count])
    return bass.AP(tensor=ap.tensor, offset=ap.offset, ap=l)


def build_program(reps=1, sq_act_mod=4, **_):
    """sq_act_mod: every sq_act_mod-th square group runs on ACT (rest DVE)."""
    nc = bacc.Bacc("TRN2", target_bir_lowering=False, debug=False)

    ptb = nc.dram_tensor("ptb", [128, NCH, IBLK, JW], BF16, kind="ExternalInput")
    sT = nc.dram_tensor("sT", [DIM, N], BF16, kind="ExternalInput")
    sTi = nc.dram_tensor("sTi", [DIM, IBLK], BF16, kind="ExternalInput")
    wq = nc.dram_tensor("wq", [DIM, DIM], BF16, kind="ExternalInput")
    wk = nc.dram_tensor("wk", [DIM, DIM], BF16, kind="ExternalInput")
    wv = nc.dram_tensor("wv", [DIM, DIM], BF16, kind="ExternalInput")
    wg = nc.dram_tensor("wg", [DIM, DIM], BF16, kind="ExternalInput")
    wo = nc.dram_tensor("wo", [DIM, DIM], BF16, kind="ExternalInput")
    w17 = nc.dram_tensor("w17", [DPAIR, 17], BF16, kind="ExternalInput")
    bgt = nc.dram_tensor("bgt", [128, 8], F32, kind="ExternalInput")
    out = nc.dram_tensor("out", [IBLK, DIM], F32, kind="ExternalOutput")

    CT = DIM // 128  # 8 contraction tiles

    with tile.TileContext(nc) as tc:
        with tc.tile_pool(name="consts", bufs=1) as consts, \
             tc.tile_pool(name="persist", bufs=1) as pers:
            ident = consts.tile([128, 128], BF16, tag="ident", name="ident")
            make_identity(nc, ident)
            ones1 = consts.tile([128, 1], BF16, tag="ones1", name="ones1")
            nc.vector.memset(ones1, 1.0)
            zero1 = consts.tile([128, 1], F32, tag="zero1", name="zero1")
            nc.vector.memset(zero1, 0.0)
            eps4 = consts.tile([128, 1], F32, tag="eps4", name="eps4")
            nc.vector.memset(eps4, EPS)
            w17_sb = consts.tile([DPAIR, 17], BF16, tag="w17", name="w17")
            nc.sync.dma_start(out=w17_sb, in_=w17[:, :])
            bgt_sb = consts.tile([128, 8], F32, tag="bgt", name="bgt")
            nc.sync.dma_start(out=bgt_sb, in_=bgt[:, :])

            for _rep in range(reps):
                # persistent tensors
                kT = [pers.tile([128, N], BF16, tag=f"kT{t}", name=f"kT{t}") for t in range(8)]
                vsb = [pers.tile([128, DIM], BF16, tag=f"v{t}", name=f"v{t}") for t in range(8)]
                qT = [pers.tile([128, IBLK], BF16, tag=f"qT{t}", name=f"qT{t}") for t in range(8)]
                gT = [pers.tile([128, IBLK], F32, tag=f"gT{t}", name=f"gT{t}") for t in range(8)]
                bias_h = pers.tile([128, HEADS, N], BF16, tag="biasH", name="biasH")
                wo_sb = pers.tile([128, CT, DIM], BF16, tag="wo", name="wo")

                with tc.tile_pool(name="pb", bufs=1) as pb, \
                     tc.tile_pool(name="psB", bufs=2, space="PSUM") as psB, \
                     tc.tile_pool(name="pa", bufs=1) as pa, \
                     tc.tile_pool(name="psA", bufs=2, space="PSUM") as psA:
                    sums = pb.tile([128, N], F32, tag="sums", name="sums")
                    sumsq = pb.tile([128, N], F32, tag="sumsq", name="sumsq")
                    rA = pb.tile([128, N], BF16, tag="rA", name="rA")

                    # ---- phase A inputs ----
                    def load8(dst, src, ncols, eng=None):
                        eng = eng or nc.sync
                        for ct in range(CT):
                            eng.dma_start(
                                out=dst[:, ct, :],
                                in_=src[ct * 128:(ct + 1) * 128, :ncols])

                    # first pairwise chunk goes out before anything else so
                    # phase B can start immediately
                    x0 = pb.tile([128, IBLK, JW], BF16, tag="x", bufs=3, name="x")
                    nc.sync.dma_start(out=x0, in_=ptb[:, 0, :, :])
                    si_sb = pa.tile([128, CT, IBLK], BF16, tag="si", name="si")
                    load8(si_sb, sTi, IBLK, nc.scalar)
                    wA = pa.tile([128, CT, DIM], BF16, tag="wA", name="wA")
                    load8(wA, wq, DIM, nc.scalar)

                    # ---- phase A unit emitters ----
                    s_sb = pa.tile([128, CT, N], BF16, tag="s", name="s")

                    def unit_q(t, eng):
                        ps = psA.tile([128, IBLK], F32, tag="mmA", name="mmA", bufs=2)
                        for ct in range(CT):
                            nc.tensor.matmul(
                                ps, wA[:, ct, t * 128:(t + 1) * 128], si_sb[:, ct, :],
                                start=(ct == 0), stop=(ct == CT - 1))
                        _copy(eng, qT[t], ps)

                    def unit_g(t, eng):
                        ps = psA.tile([128, IBLK], F32, tag="mmA", name="mmA", bufs=2)
                        for ct in range(CT):
                            nc.tensor.matmul(
                                ps, wB[:, ct, t * 128:(t + 1) * 128], si_sb[:, ct, :],
                                start=(ct == 0), stop=(ct == CT - 1))
                        nc.scalar.activation(out=gT[t], in_=ps, func=AF.Sigmoid,
                                             bias=bgt_sb[:, t:t + 1], scale=1.0)

                    def unit_k(t, jh, eng):
                        ps = psA.tile([128, 512], F32, tag="mmA2", name="mmA2", bufs=2)
                        for ct in range(CT):
                            nc.tensor.matmul(
                                ps, wA[:, ct, t * 128:(t + 1) * 128],
                                s_sb[:, ct, jh * 512:(jh + 1) * 512],
                                start=(ct == 0), stop=(ct == CT - 1))
                        _copy(eng, kT[t][:, jh * 512:(jh + 1) * 512], ps)

                    def unit_v(t, vh, eng):
                        ps = psA.tile([128, 512], F32, tag="mmA2", name="mmA2", bufs=2)
                        for ct in range(CT):
                            nc.tensor.matmul(
                                ps, s_sb[:, ct, t * 128:(t + 1) * 128],
                                wB[:, ct, vh * 512:(vh + 1) * 512],
                                start=(ct == 0), stop=(ct == CT - 1))
                        _copy(eng, vsb[t][:, vh * 512:(vh + 1) * 512], ps)

                    def _copy(eng, dst, src):
                        # PSUM sources: only DVE/ACT may touch PSUM
                        if eng % 2 == 0:
                            nc.vector.tensor_copy(out=dst, in_=src)
                        else:
                            nc.scalar.copy(out=dst, in_=src)

                    units = []
                    for t in range(8):
                        units.append(lambda t=t: unit_q(t, t % 2))
                    for t in range(8):
                        units.append(lambda t=t: unit_g(t, t % 2))
                    for t in range(8):
                        units.append(lambda t=t: unit_k(t, 0, 0))
                        units.append(lambda t=t: unit_k(t, 1, 1))
                    for t in range(8):
                        units.append(lambda t=t: unit_v(t, 0, 0))
                        units.append(lambda t=t: unit_v(t, 1, 1))
                    n_units = len(units)  # 48
                    unit_idx = 0

                    # emission checkpoints: after chunk jb, how many units
                    # should have been emitted (q/g early, k/v from chunk 3)
                    quota = [(jb + 1) * 16 // 6 if jb < 6 else
                             16 + ((jb - 5) * 32 + 25) // 26
                             for jb in range(NCH)]
                    quota[-1] = n_units

                    def emit_stats(j0, j1, on_pool=True):
                        # stats post-pass runs on Pool (SBUF-only data);
                        # the final batch uses DVE for lower chain latency
                        sl = slice(j0, j1)
                        w = j1 - j0
                        seng = nc.gpsimd if on_pool else nc.vector
                        mu = pb.tile([128, 256], F32, tag="mu", name="mu", bufs=2)[:, :w]
                        v4 = pb.tile([128, 256], F32, tag="v4", name="v4", bufs=2)[:, :w]
                        d = pb.tile([128, 256], F32, tag="d", name="d", bufs=2)[:, :w]
                        seng.tensor_scalar_mul(out=mu, in0=sums[:, sl],
                                               scalar1=1.0 / DPAIR)
                        seng.tensor_scalar_mul(out=v4, in0=sumsq[:, sl],
                                               scalar1=1.0 / DPAIR)
                        seng.tensor_mul(out=d, in0=mu, in1=mu)
                        seng.tensor_sub(out=v4, in0=v4, in1=d)
                        nc.scalar.activation(out=v4, in_=v4, func=AF.Sqrt,
                                             bias=eps4[:, 0:1], scale=1.0)
                        with nc.allow_low_precision(reason="rsqrt to bf16 is plenty for a softmax bias"):
                            nc.vector.reciprocal(out=rA[:, sl], in_=v4)
                        rA_b = _insert_bcast(rA[:, sl], HEADS, 0)
                        nc.vector.tensor_mul(out=bias_h[:, :, sl],
                                             in0=bias_h[:, :, sl], in1=rA_b)

                    # ---- phase B chunk loop with A units interleaved ----
                    for jb in range(NCH):
                        if jb == 0:
                            x = x0
                        else:
                            x = pb.tile([128, IBLK, JW], BF16, tag="x", bufs=3, name="x")
                            nc.sync.dma_start(out=x, in_=ptb[:, jb, :, :])
                        # stagger the remaining input DMAs between chunks
                        if jb == 0:
                            wB = pa.tile([128, CT, DIM], BF16, tag="wB", name="wB")
                            load8(wB, wg, DIM, nc.scalar)
                        elif jb == 1:
                            load8(s_sb, sT, N, nc.scalar)
                        elif jb == 4:
                            load8(wA, wk, DIM)
                        elif jb == 8:
                            load8(wB, wv, DIM)
                        elif jb == 12:
                            load8(wo_sb, wo, DIM, nc.scalar)

                        # squares first (rotated DVE/ACT/Pool) so sq matmuls
                        # aren't waiting when PE reaches them
                        xs_t = []
                        for g in range(JW // JG):
                            j0 = g * JG
                            xs = pb.tile([128, IBLK, JG], BF16, tag="xsq",
                                         bufs=4, name="xsq")
                            xs_t.append(xs)
                            gi = (jb * (JW // JG) + g) % 16
                            if gi in (0, 6, 12):
                                nc.scalar.activation(out=xs, in_=x[:, :, j0:j0 + JG],
                                                     func=AF.Square)
                            elif gi in (2, 8, 14):
                                nc.gpsimd.tensor_mul(out=xs, in0=x[:, :, j0:j0 + JG],
                                                     in1=x[:, :, j0:j0 + JG])
                            else:
                                nc.vector.tensor_mul(out=xs, in0=x[:, :, j0:j0 + JG],
                                                     in1=x[:, :, j0:j0 + JG])
                        for g in range(JW // JG):
                            j0 = jb * JW + g * JG
                            gi = jb * (JW // JG) + g
                            pp = psB.tile([128, JG, 17], F32, tag="pp", bufs=2,
                                          name="pp")
                            for jj in range(JG):
                                nc.tensor.matmul(pp[:, jj, :], x[:, :, g * JG + jj],
                                                 w17_sb, start=True, stop=True)
                            sq = psB.tile([128, JG], F32, tag="sq", bufs=2, name="sq")
                            for jj in range(JG):
                                nc.tensor.matmul(sq[:, jj:jj + 1], xs_t[g][:, :, jj],
                                                 ones1, start=True, stop=True)
                            # copies: bias block + stats rotate DVE/ACT
                            bsl = bias_h[:, :, j0:j0 + JG]
                            _copy(gi % 2, _swap_free(bsl), pp[:, :, 0:16])
                            _copy(gi + 1, sums[:, j0:j0 + JG], pp[:, :, 16])
                            _copy(gi + 1, sumsq[:, j0:j0 + JG], sq)

                        if jb == NCH - 3:
                            # preload the Exp act table before the tail chain
                            dummy = pb.tile([128, 1], F32, tag="dummy", name="dummy")
                            nc.scalar.activation(out=dummy, in_=eps4, func=AF.Exp,
                                                 bias=zero1[:, 0:1], scale=1.0)
                        if jb % (NCH // 4) == NCH // 4 - 1 and jb != NCH - 1:
                            q = jb // (NCH // 4)
                            emit_stats(q * 256, q * 256 + 256)
                        elif jb == NCH - 2:
                            emit_stats(768, 896)
                        elif jb == NCH - 1:
                            emit_stats(896, 1024, on_pool=False)
                        while unit_idx < quota[jb]:
                            units[unit_idx]()
                            unit_idx += 1

                # ---------------- Phase C: attention ------------------------
                with tc.tile_pool(name="pc", bufs=1) as pc, \
                     tc.tile_pool(name="psC", bufs=2, space="PSUM") as psC:
                    og = [pc.tile([128, IBLK], BF16, tag=f"og{t}", name=f"og{t}") for t in range(8)]

                    # Software-pipelined head loop: head h's scores+softmax are
                    # emitted one iteration ahead of head h's attn@v, so the
                    # PE never sits idle waiting for exp/normalize and stays
                    # out of the low p-state.
                    ot_ps = {}
                    attn_t = {}

                    def emit_scores(h):
                        t = h // 2
                        off = 64 * (h % 2)
                        # bias pre-written into PSUM via an identity matmul;
                        # the qk matmul accumulates on top with start=False,
                        # and exp reads straight from PSUM — no separate add.
                        ps = psC.tile([128, N], F32, tag="sc", bufs=2, name="sc")
                        for jh in range(2):
                            nc.tensor.matmul(
                                ps[:, jh * 512:(jh + 1) * 512], ident,
                                bias_h[:, h, jh * 512:(jh + 1) * 512],
                                start=True, stop=False, skip_group_check=True)
                            nc.tensor.matmul(
                                ps[:, jh * 512:(jh + 1) * 512],
                                qT[t][off:off + 64, :],
                                kT[t][off:off + 64, jh * 512:(jh + 1) * 512],
                                start=False, stop=True, skip_group_check=True)
                        # scores are O(10) here: exp() without max-subtraction
                        # is safe in f32 and softmax is shift-invariant.
                        ssum = pc.tile([128, 1], F32, tag="ssum", bufs=3, name="ssum")
                        attn = pc.tile([128, N], BF16, tag="attn", bufs=3, name="attn")
                        nc.scalar.activation(out=attn, in_=ps, func=AF.Exp,
                                             bias=zero1[:, 0:1], scale=1.0,
                                             accum_out=ssum)
                        rs = pc.tile([128, 1], F32, tag="rs", bufs=3, name="rs")
                        nc.vector.reciprocal(out=rs, in_=ssum)
                        nc.vector.tensor_scalar_mul(out=attn, in0=attn, scalar1=rs)
                        attn_t[h] = attn

                    def emit_av(h):
                        t = h // 2
                        off = 64 * (h % 2)
                        attn = attn_t.pop(h)
                        if h % 2 == 0:
                            ot_ps[t] = psC.tile([128, IBLK], F32, tag="ot", bufs=1, name="ot")
                        for j4 in range(2):
                            pT = psC.tile([128, 4, 128], BF16, tag="pT", bufs=2, name="pT")
                            for jj in range(4):
                                jt = j4 * 4 + jj
                                nc.tensor.transpose(
                                    pT[:, jj, :],
                                    attn[:, jt * 128:(jt + 1) * 128], ident)
                            aT = pc.tile([128, 4, 128], BF16, tag="aT", bufs=4, name="aT")
                            if (h + j4) % 2 == 0:
                                nc.vector.tensor_copy(out=aT.bitcast(F32),
                                                      in_=pT.bitcast(F32))
                            else:
                                nc.scalar.copy(out=aT.bitcast(F32),
                                               in_=pT.bitcast(F32))
                            for jj in range(4):
                                jt = j4 * 4 + jj
                                nc.tensor.matmul(
                                    ot_ps[t][off:off + 64, :],
                                    vsb[jt][:, h * 64:(h + 1) * 64], aT[:, jj, :],
                                    start=(jt == 0), stop=(jt == 7))
                        if h % 2 == 1:
                            nc.vector.tensor_mul(out=og[t], in0=ot_ps.pop(t), in1=gT[t])

                    for h in range(HEADS + 1):
                        if h < HEADS:
                            emit_scores(h)
                        if h >= 1:
                            emit_av(h - 1)

                    # out = og^T @ Wo
                    out_sb = pc.tile([128, DIM], F32, tag="out_sb", name="out_sb")
                    for eh in range(2):
                        ps = psC.tile([128, 512], F32, tag="po", bufs=1, name="po")
                        for t in range(8):
                            nc.tensor.matmul(
                                ps, og[t], wo_sb[:, t, eh * 512:(eh + 1) * 512],
                                start=(t == 0), stop=(t == 7))
                        nc.scalar.copy(out=out_sb[:, eh * 512:(eh + 1) * 512], in_=ps)
                    nc.sync.dma_start(out=out[:, :], in_=out_sb)

    nc.compile()
    return nc


_CACHE = {}


def _prep_inputs(single_repr, pairwise_repr, ln_gamma, ln_beta, W_bias,
                 Wq, Wk, Wv, Wg, bg, Wo):
    sr = np.asarray(single_repr, np.float32).reshape(N, DIM)
    pw = np.asarray(pairwise_repr, np.float32).reshape(N, N, DPAIR)
    gamma = np.asarray(ln_gamma, np.float32)
    Wb = np.asarray(W_bias, np.float32)
    weff = gamma[:, None] * Wb                                   # [128, 16]
    wp = weff - weff.sum(0, keepdims=True) / DPAIR               # mean folded
    w17 = np.concatenate([wp, np.ones((DPAIR, 1), np.float32)], axis=1)
    sT_np = np.ascontiguousarray(sr.T).astype(BFNP)              # [DIM, N]
    scale = DHEAD ** -0.5
    pwb = pw.astype(BFNP)                                        # [N, N, 128]
    common = {
        "sT": sT_np,
        "wq": (np.asarray(Wq, np.float32) * scale).astype(BFNP),
        "wk": np.asarray(Wk, np.float32).astype(BFNP),
        "wv": np.asarray(Wv, np.float32).astype(BFNP),
        "wg": np.asarray(Wg, np.float32).astype(BFNP),
        "wo": np.asarray(Wo, np.float32).astype(BFNP),
        "w17": w17.astype(BFNP),
        "bgt": np.ascontiguousarray(
            np.asarray(bg, np.float32).reshape(8, 128).T),
    }
    in_maps = []
    for c in range(NCORES):
        m = dict(common)
        # [i, j, p] -> [p, jb, i, jw]
        blk = pwb[c * IBLK:(c + 1) * IBLK]                       # [128i, N, 128p]
        m["ptb"] = np.ascontiguousarray(
            blk.transpose(2, 0, 1).reshape(128, IBLK, NCH, JW)
               .transpose(0, 2, 1, 3))
        m["sTi"] = np.ascontiguousarray(sT_np[:, c * IBLK:(c + 1) * IBLK])
        in_maps.append(m)
    return in_maps


def kernel(single_repr, pairwise_repr, ln_gamma, ln_beta, W_bias,
           Wq, Wk, Wv, Wg, bg, Wo, _trace=False):
    if "nc" not in _CACHE:
        _CACHE["nc"] = build_program()
    nc = _CACHE["nc"]
    in_maps = _prep_inputs(single_repr, pairwise_repr, ln_gamma, ln_beta,
                           W_bias, Wq, Wk, Wv, Wg, bg, Wo)
    res = run_bass_kernel_spmd(nc, in_maps, core_ids=list(range(NCORES)),
                               trace=_trace)
    out = np.concatenate([res.results[c]["out"] for c in range(NCORES)], axis=0)
    if _trace:
        kernel.last_result = res
    return out.reshape(1, N, DIM).astype(np.float32)


# revision 35
# speedup vs baseline: 1.3004x; 1.0462x over previous
"""AttentionPairBias kernel for Trainium2, 8-core SPMD — v2.

Math (per batch=1):
  pn        = LayerNorm(pairwise) * gamma + beta                  [N, N, 128]
  attn_bias = einsum('ijp,ph->hij', pn, W_bias)                   [16, N, N]
  q,k,v     = single @ Wq/Wk/Wv  (split into 16 heads of 64)
  scores    = q k^T / sqrt(64) + attn_bias ; attn = softmax_j
  o         = attn @ v ; out = (o * sigmoid(single@Wg + bg)) @ Wo [N, 1024]

Sharding: rows of i (queries) across 8 cores; k/v compute replicated.

v2 design vs v1:
- The pairwise tensor is pre-transposed ON HOST to [p, i, j] layout (bf16),
  removing all on-chip PE transposes and the 16.7M-element PSUM->SBUF
  copies of v1. DMA ships 32MB/core instead of 64MB-read.
- LayerNorm mean correction is folded into the weights:
      W' = gamma*W_bias - colsum(gamma*W_bias)/128
  so bias = rsqrt(var+eps) * (x @ W'), turning the per-element affine
  correction into a single broadcast multiply.
- Per j-column: matmul(x_tile[p,i] stationary, [W'|ones] moving) gives
  bias products AND row-sums in one shot; a second 1-column matmul with a
  squared copy of the tile gives sumsq (for variance).
- softmax 1/sum scaling folded into the attention transpose by using
  diag(1/sum) instead of the identity matrix.
- qkvg projections interleaved between pairwise chunks so the PE stays
  busy while DMA streams the big tensor; all DMAs issued from SP.
"""

import numpy as np
import ml_dtypes

import concourse.bacc as bacc
import concourse.bass as bass
import concourse.tile as tile
import concourse.mybir as mybir
from concourse.bass_utils import run_bass_kernel_spmd
from concourse.masks import make_identity

N, DIM, HEADS, DHEAD, DPAIR = 1024, 1024, 16, 64, 128
NCORES = 8
IBLK = N // NCORES  # 128
EPS = 1e-5
JW = 32            # j-columns per pairwise DMA chunk
NCH = N // JW      # 32 chunks
JG = 16            # j-columns per PSUM group

F32 = mybir.dt.float32
BF16 = mybir.dt.bfloat16
AX = mybir.AxisListType
AF = mybir.ActivationFunctionType
BFNP = ml_dtypes.bfloat16


def _swap_free(ap):
    """Swap the two free dims of a 3D AP (iteration-order change)."""
    l = list(ap.ap)
    assert len(l) == 3
    return bass.AP(tensor=ap.tensor, offset=ap.offset, ap=[l[0], l[2], l[1]])


def _insert_bcast(ap, count, pos):
    """Insert a zero-stride broadcast dim of length `count` at free-dim
    position `pos` (0 = right after the partition dim)."""
    l = list(ap.ap)
    l.insert(1 + pos, [0, count])
    return bass.AP(tensor=ap.tensor, offset=ap.offset, ap=l)


def build_program(reps=1, sq_act_mod=4, **_):
    """sq_act_mod: every sq_act_mod-th square group runs on ACT (rest DVE)."""
    nc = bacc.Bacc("TRN2", target_bir_lowering=False, debug=False)

    ptb = nc.dram_tensor("ptb", [128, NCH, IBLK, JW], BF16, kind="ExternalInput")
    sT = nc.dram_tensor("sT", [DIM, N], BF16, kind="ExternalInput")
    sTi = nc.dram_tensor("sTi", [DIM, IBLK], BF16, kind="ExternalInput")
    wq = nc.dram_tensor("wq", [DIM, DIM], BF16, kind="ExternalInput")
    wk = nc.dram_tensor("wk", [DIM, DIM], BF16, kind="ExternalInput")
    wv = nc.dram_tensor("wv", [DIM, DIM], BF16, kind="ExternalInput")
    wg = nc.dram_tensor("wg", [DIM, DIM], BF16, kind="ExternalInput")
    wo = nc.dram_tensor("wo", [DIM, DIM], BF16, kind="ExternalInput")
    w17 = nc.dram_tensor("w17", [DPAIR, 17], BF16, kind="ExternalInput")
    bgt = nc.dram_tensor("bgt", [128, 8], F32, kind="ExternalInput")
    out = nc.dram_tensor("out", [IBLK, DIM], F32, kind="ExternalOutput")

    CT = DIM // 128  # 8 contraction tiles

    with tile.TileContext(nc) as tc:
        with tc.tile_pool(name="consts", bufs=1) as consts, \
             tc.tile_pool(name="persist", bufs=1) as pers:
            ident = consts.tile([128, 128], BF16, tag="ident", name="ident")
            make_identity(nc, ident)
            ones1 = consts.tile([128, 1], BF16, tag="ones1", name="ones1")
            nc.vector.memset(ones1, 1.0)
            zero1 = consts.tile([128, 1], F32, tag="zero1", name="zero1")
            nc.vector.memset(zero1, 0.0)
            eps4 = consts.tile([128, 1], F32, tag="eps4", name="eps4")
            nc.vector.memset(eps4, EPS)
            w17_sb = consts.tile([DPAIR, 17], BF16, tag="w17", name="w17")
            nc.sync.dma_start(out=w17_sb, in_=w17[:, :])
            bgt_sb = consts.tile([128, 8], F32, tag="bgt", name="bgt")
            nc.sync.dma_start(out=bgt_sb, in_=bgt[:, :])

            for _rep in range(reps):
                # persistent tensors
                kT = [pers.tile([128, N], BF16, tag=f"kT{t}", name=f"kT{t}") for t in range(8)]
                vsb = [pers.tile([128, DIM], BF16, tag=f"v{t}", name=f"v{t}") for t in range(8)]
                qT = [pers.tile([128, IBLK], BF16, tag=f"qT{t}", name=f"qT{t}") for t in range(8)]
                gT = [pers.tile([128, IBLK], F32, tag=f"gT{t}", name=f"gT{t}") for t in range(8)]
                bias_h = pers.tile([128, HEADS, N], BF16, tag="biasH", name="biasH")
                wo_sb = pers.tile([128, CT, DIM], BF16, tag="wo", name="wo")

                with tc.tile_pool(name="pb", bufs=1) as pb, \
                     tc.tile_pool(name="psB", bufs=2, space="PSUM") as psB, \
                     tc.tile_pool(name="pa", bufs=1) as pa, \
                     tc.tile_pool(name="psA", bufs=2, space="PSUM") as psA:
                    sums = pb.tile([128, N], F32, tag="sums", name="sums")
                    sumsq = pb.tile([128, N], F32, tag="sumsq", name="sumsq")
                    rA = pb.tile([128, N], BF16, tag="rA", name="rA")

                    # ---- phase A inputs ----
                    def load8(dst, src, ncols, eng=None):
                        eng = eng or nc.sync
                        for ct in range(CT):
                            eng.dma_start(
                                out=dst[:, ct, :],
                                in_=src[ct * 128:(ct + 1) * 128, :ncols])

                    # first pairwise chunk goes out before anything else so
                    # phase B can start immediately
                    x0 = pb.tile([128, IBLK, JW], BF16, tag="x", bufs=3, name="x")
                    nc.sync.dma_start(out=x0, in_=ptb[:, 0, :, :])
                    si_sb = pa.tile([128, CT, IBLK], BF16, tag="si", name="si")
                    load8(si_sb, sTi, IBLK, nc.scalar)
                    wA = pa.tile([128, CT, DIM], BF16, tag="wA", name="wA")
                    load8(wA, wq, DIM, nc.scalar)

                    # ---- phase A unit emitters ----
                    s_sb = pa.tile([128, CT, N], BF16, tag="s", name="s")

                    def unit_q(t, eng):
                        ps = psA.tile([128, IBLK], F32, tag="mmA", name="mmA", bufs=2)
                        for ct in range(CT):
                            nc.tensor.matmul(
                                ps, wA[:, ct, t * 128:(t + 1) * 128], si_sb[:, ct, :],
                                start=(ct == 0), stop=(ct == CT - 1))
                        _copy(eng, qT[t], ps)

                    def unit_g(t, eng):
                        ps = psA.tile([128, IBLK], F32, tag="mmA", name="mmA", bufs=2)
                        for ct in range(CT):
                            nc.tensor.matmul(
                                ps, wB[:, ct, t * 128:(t + 1) * 128], si_sb[:, ct, :],
                                start=(ct == 0), stop=(ct == CT - 1))
                        nc.scalar.activation(out=gT[t], in_=ps, func=AF.Sigmoid,
                                             bias=bgt_sb[:, t:t + 1], scale=1.0)

                    def unit_k(t, jh, eng):
                        ps = psA.tile([128, 512], F32, tag="mmA2", name="mmA2", bufs=2)
                        for ct in range(CT):
                            nc.tensor.matmul(
                                ps, wA[:, ct, t * 128:(t + 1) * 128],
                                s_sb[:, ct, jh * 512:(jh + 1) * 512],
                                start=(ct == 0), stop=(ct == CT - 1))
                        _copy(eng, kT[t][:, jh * 512:(jh + 1) * 512], ps)

                    def unit_v(t, vh, eng):
                        ps = psA.tile([128, 512], F32, tag="mmA2", name="mmA2", bufs=2)
                        for ct in range(CT):
                            nc.tensor.matmul(
                                ps, s_sb[:, ct, t * 128:(t + 1) * 128],
                                wB[:, ct, vh * 512:(vh + 1) * 512],
                                start=(ct == 0), stop=(ct == CT - 1))
                        _copy(eng, vsb[t][:, vh * 512:(vh + 1) * 512], ps)

                    def _copy(eng, dst, src):
                        # PSUM sources: only DVE/ACT may touch PSUM
                        if eng % 2 == 0:
                            nc.vector.tensor_copy(out=dst, in_=src)
                        else:
                            nc.scalar.copy(out=dst, in_=src)

                    units = []
                    for t in range(8):
                        units.append(lambda t=t: unit_q(t, t % 2))
                    for t in range(8):
                        units.append(lambda t=t: unit_g(t, t % 2))
                    for t in range(8):
                        units.append(lambda t=t: unit_k(t, 0, 0))
                        units.append(lambda t=t: unit_k(t, 1, 1))
                    for t in range(8):
                        units.append(lambda t=t: unit_v(t, 0, 0))
                        units.append(lambda t=t: unit_v(t, 1, 1))
                    n_units = len(units)  # 48
                    unit_idx = 0

                    # emission checkpoints: after chunk jb, how many units
                    # should have been emitted (q/g early, k/v from chunk 3)
                    quota = [(jb + 1) * 16 // 6 if jb < 6 else
                             16 + ((jb - 5) * 32 + 25) // 26
                             for jb in range(NCH)]
                    quota[-1] = n_units

                    def emit_stats(j0, j1, on_pool=True):
                        # stats post-pass runs on Pool (SBUF-only data);
                        # the final batch uses DVE for lower chain latency
                        sl = slice(j0, j1)
                        w = j1 - j0
                        seng = nc.gpsimd if on_pool else nc.vector
                        mu = pb.tile([128, 256], F32, tag="mu", name="mu", bufs=2)[:, :w]
                        v4 = pb.tile([128, 256], F32, tag="v4", name="v4", bufs=2)[:, :w]
                        d = pb.tile([128, 256], F32, tag="d", name="d", bufs=2)[:, :w]
                        seng.tensor_scalar_mul(out=mu, in0=sums[:, sl],
                                               scalar1=1.0 / DPAIR)
                        seng.tensor_scalar_mul(out=v4, in0=sumsq[:, sl],
                                               scalar1=1.0 / DPAIR)
                        seng.tensor_mul(out=d, in0=mu, in1=mu)
                        seng.tensor_sub(out=v4, in0=v4, in1=d)
                        nc.scalar.activation(out=v4, in_=v4, func=AF.Sqrt,
                                             bias=eps4[:, 0:1], scale=1.0)
                        with nc.allow_low_precision(reason="rsqrt to bf16 is plenty for a softmax bias"):
                            nc.vector.reciprocal(out=rA[:, sl], in_=v4)
                        rA_b = _insert_bcast(rA[:, sl], HEADS, 0)
                        nc.vector.tensor_mul(out=bias_h[:, :, sl],
                                             in0=bias_h[:, :, sl], in1=rA_b)

                    # ---- phase B chunk loop with A units interleaved ----
                    for jb in range(NCH):
                        if jb == 0:
                            x = x0
                        else:
                            x = pb.tile([128, IBLK, JW], BF16, tag="x", bufs=3, name="x")
                            nc.sync.dma_start(out=x, in_=ptb[:, jb, :, :])
                        # stagger the remaining input DMAs between chunks
                        if jb == 0:
                            wB = pa.tile([128, CT, DIM], BF16, tag="wB", name="wB")
                            load8(wB, wg, DIM, nc.scalar)
                        elif jb == 1:
                            load8(s_sb, sT, N, nc.scalar)
                        elif jb == 4:
                            load8(wA, wk, DIM)
                        elif jb == 8:
                            load8(wB, wv, DIM)
                        elif jb == 12:
                            load8(wo_sb, wo, DIM, nc.scalar)

                        # squares first (rotated DVE/ACT/Pool) so sq matmuls
                        # aren't waiting when PE reaches them
                        xs_t = []
                        for g in range(JW // JG):
                            j0 = g * JG
                            xs = pb.tile([128, IBLK, JG], BF16, tag="xsq",
                                         bufs=4, name="xsq")
                            xs_t.append(xs)
                            gi = (jb * (JW // JG) + g) % 16
                            if gi in (0, 6, 12):
                                nc.scalar.activation(out=xs, in_=x[:, :, j0:j0 + JG],
                                                     func=AF.Square)
                            elif gi in (2, 8, 14):
                                nc.gpsimd.tensor_mul(out=xs, in0=x[:, :, j0:j0 + JG],
                                                     in1=x[:, :, j0:j0 + JG])
                            else:
                                nc.vector.tensor_mul(out=xs, in0=x[:, :, j0:j0 + JG],
                                                     in1=x[:, :, j0:j0 + JG])
                        for g in range(JW // JG):
                            j0 = jb * JW + g * JG
                            gi = jb * (JW // JG) + g
                            pp = psB.tile([128, JG, 17], F32, tag="pp", bufs=2,
                                          name="pp")
                            for jj in range(JG):
                                nc.tensor.matmul(pp[:, jj, :], x[:, :, g * JG + jj],
                                                 w17_sb, start=True, stop=True)
                            sq = psB.tile([128, JG], F32, tag="sq", bufs=2, name="sq")
                            for jj in range(JG):
                                nc.tensor.matmul(sq[:, jj:jj + 1], xs_t[g][:, :, jj],
                                                 ones1, start=True, stop=True)
                            # copies: bias block + stats rotate DVE/ACT
                            bsl = bias_h[:, :, j0:j0 + JG]
                            _copy(gi % 2, _swap_free(bsl), pp[:, :, 0:16])
                            _copy(gi + 1, sums[:, j0:j0 + JG], pp[:, :, 16])
                            _copy(gi + 1, sumsq[:, j0:j0 + JG], sq)

                        if jb == NCH - 3:
                            # preload the Exp act table before the tail chain
                            dummy = pb.tile([128, 1], F32, tag="dummy", name="dummy")
                            nc.scalar.activation(out=dummy, in_=eps4, func=AF.Exp,
                                                 bias=zero1[:, 0:1], scale=1.0)
                        if jb % (NCH // 4) == NCH // 4 - 1 and jb != NCH - 1:
                            q = jb // (NCH // 4)
                            emit_stats(q * 256, q * 256 + 256)
                        elif jb == NCH - 2:
                            emit_stats(768, 896)
                        elif jb == NCH - 1:
                            emit_stats(896, 1024, on_pool=False)
                        while unit_idx < quota[jb]:
                            units[unit_idx]()
                            unit_idx += 1

                # ---------------- Phase C: attention ------------------------
                with tc.tile_pool(name="pc", bufs=1) as pc, \
                     tc.tile_pool(name="psC", bufs=2, space="PSUM") as psC:
                    og = [pc.tile([128, IBLK], BF16, tag=f"og{t}", name=f"og{t}") for t in range(8)]

                    # Software-pipelined head loop: head h's scores+softmax are
                    # emitted one iteration ahead of head h's attn@v, so the
                    # PE never sits idle waiting for exp/normalize and stays
                    # out of the low p-state.
                    ot_ps = {}
                    attn_t = {}

                    def emit_scores(h):
                        t = h // 2
                        off = 64 * (h % 2)
                        # bias pre-written into PSUM via an identity matmul;
                        # the qk matmul accumulates on top with start=False,
                        # and exp reads straight from PSUM — no separate add.
                        ps = psC.tile([128, N], F32, tag="sc", bufs=2, name="sc")
                        for jh in range(2):
                            nc.tensor.matmul(
                                ps[:, jh * 512:(jh + 1) * 512], ident,
                                bias_h[:, h, jh * 512:(jh + 1) * 512],
                                start=True, stop=False, skip_group_check=True)
                            nc.tensor.matmul(
                                ps[:, jh * 512:(jh + 1) * 512],
                                qT[t][off:off + 64, :],
                                kT[t][off:off + 64, jh * 512:(jh + 1) * 512],
                                start=False, stop=True, skip_group_check=True)
                        # scores are O(10) here: exp() without max-subtraction
                        # is safe in f32 and softmax is shift-invariant.
                        ssum = pc.tile([128, 1], F32, tag="ssum", bufs=3, name="ssum")
                        attn = pc.tile([128, N], BF16, tag="attn", bufs=3, name="attn")
                        nc.scalar.activation(out=attn, in_=ps, func=AF.Exp,
                                             bias=zero1[:, 0:1], scale=1.0,
                                             accum_out=ssum)
                        rs = pc.tile([128, 1], F32, tag="rs", bufs=3, name="rs")
                        nc.vector.reciprocal(out=rs, in_=ssum)
                        nc.vector.tensor_scalar_mul(out=attn, in0=attn, scalar1=rs)
                        attn_t[h] = attn

                    def emit_av(h):
                        t = h // 2
                        off = 64 * (h % 2)
                        attn = attn_t.pop(h)
                        if h % 2 == 0:
                            ot_ps[t] = psC.tile([128, IBLK], F32, tag="ot", bufs=1, name="ot")
                        for j4 in range(2):
                            pT = psC.tile([128, 4, 128], BF16, tag="pT", bufs=2, name="pT")
                            for jj in range(4):
                                jt = j4 * 4 + jj
                                nc.tensor.transpose(
                                    pT[:, jj, :],
                                    attn[:, jt * 128:(jt + 1) * 128], ident)
                            aT = pc.tile([128, 4, 128], BF16, tag="aT", bufs=4, name="aT")
                            if (h + j4) % 2 == 0:
                                nc.vector.tensor_copy(out=aT.bitcast(F32),
                                                      in_=pT.bitcast(F32))
                            else:
                                nc.scalar.copy(out=aT.bitcast(F32),
                                               in_=pT.bitcast(F32))
                            for jj in range(4):
                                jt = j4 * 4 + jj
                                nc.tensor.matmul(
                                    ot_ps[t][off:off + 64, :],
                                    vsb[jt][:, h * 64:(h + 1) * 64], aT[:, jj, :],
                                    start=(jt == 0), stop=(jt == 7))
                        if h % 2 == 1:
                            nc.vector.tensor_mul(out=og[t], in0=ot_ps.pop(t), in1=gT[t])

                    for h in range(HEADS + 1):
                        if h < HEADS:
                            emit_scores(h)
                        if h >= 1:
                            emit_av(h - 1)

                    # out = og^T @ Wo
                    out_sb = pc.tile([128, DIM], F32, tag="out_sb", name="out_sb")
                    for eh in range(2):
                        ps = psC.tile([128, 512], F32, tag="po", bufs=1, name="po")
                        for t in range(8):
                            nc.tensor.matmul(
                                ps, og[t], wo_sb[:, t, eh * 512:(eh + 1) * 512],
                                start=(t == 0), stop=(t == 7))
                        nc.scalar.copy(out=out_sb[:, eh * 512:(eh + 1) * 512], in_=ps)
                    nc.sync.dma_start(out=out[:, :], in_=out_sb)

    nc.compile()
    return nc


_CACHE = {}


def _prep_inputs(single_repr, pairwise_repr, ln_gamma, ln_beta, W_bias,
                 Wq, Wk, Wv, Wg, bg, Wo):
    sr = np.asarray(single_repr, np.float32).reshape(N, DIM)
    pw = np.asarray(pairwise_repr, np.float32).reshape(N, N, DPAIR)
    gamma = np.asarray(ln_gamma, np.float32)
    Wb = np.asarray(W_bias, np.float32)
    weff = gamma[:, None] * Wb                                   # [128, 16]
    wp = weff - weff.sum(0, keepdims=True) / DPAIR               # mean folded
    w17 = np.concatenate([wp, np.ones((DPAIR, 1), np.float32)], axis=1)
    sT_np = np.ascontiguousarray(sr.T).astype(BFNP)              # [DIM, N]
    scale = DHEAD ** -0.5
    pwb = pw.astype(BFNP)                                        # [N, N, 128]
    common = {
        "sT": sT_np,
        "wq": (np.asarray(Wq, np.float32) * scale).astype(BFNP),
        "wk": np.asarray(Wk, np.float32).astype(BFNP),
        "wv": np.asarray(Wv, np.float32).astype(BFNP),
        "wg": np.asarray(Wg, np.float32).astype(BFNP),
        "wo": np.asarray(Wo, np.float32).astype(BFNP),
        "w17": w17.astype(BFNP),
        "bgt": np.ascontiguousarray(
            np.asarray(bg, np.float32).reshape(8, 128).T),
    }
    in_maps = []
    for c in range(NCORES):
        m = dict(common)
        # [i, j, p] -> [p, jb, i, jw]
        blk = pwb[c * IBLK:(c + 1) * IBLK]                       # [128i, N, 128p]
        m["ptb"] = np.ascontiguousarray(
            blk.transpose(2, 0, 1).reshape(128, IBLK, NCH, JW)
               .transpose(0, 2, 1, 3))
        m["sTi"] = np.ascontiguousarray(sT_np[:, c * IBLK:(c + 1) * IBLK])
        in_maps.append(m)
    return in_maps


def kernel(single_repr, pairwise_repr, ln_gamma, ln_beta, W_bias,
           Wq, Wk, Wv, Wg, bg, Wo, _trace=False):
    if "nc" not in _CACHE:
        _CACHE["nc"] = build_program()
    nc = _CACHE["nc"]
    in_maps = _prep_inputs(single_repr, pairwise_repr, ln_gamma, ln_beta,
                           W_bias, Wq, Wk, Wv, Wg, bg, Wo)
    res = run_bass_kernel_spmd(nc, in_maps, core_ids=list(range(NCORES)),
                               trace=_trace)
    out = np.concatenate([res.results[c]["out"] for c in range(NCORES)], axis=0)
    if _trace:
        kernel.last_result = res
    return out.reshape(1, N, DIM).astype(np.float32)
